# revision 58
# baseline (speedup 1.0000x reference)
"""Trainium2 Bass kernel for GQA attention (B=4, S=2048, D=768, H=12, KVH=4, HD=64).

Sharding: core = (batch, head-half). Each core computes 6 query heads and its
2 KV heads over the full sequence for one batch; the wo projection produces a
partial [768, 2048] output (6 of 12 head contributions) and the host adds the
two halves per batch. No collectives; K/V projection work is fully partitioned
across cores (no duplication).

Attention uses exact causal tiling with 128-row query chunks: chunk j2 attends
key tiles 0..j2, so no fully-masked tiles are ever computed; only the diagonal
tile needs a (single, constant) triangular mask, applied post-exp as a DVE
multiply. For the last chunk the diagonal tile is processed FIRST (the PV
accumulation is commutative) so that multiply leaves the serial tail chain.

The softmax denominator is folded into the PV matmul: V tiles carry an
all-ones column at position 0, so the PV output row 0 accumulates sum(exp) for
free and lands on PSUM partition 0. Normalization: write row 0 to DRAM, read
it back replicated across 65 partitions (DRAM replicated reads are fast; SBUF
single-partition replication, gpsimd partition_broadcast, and swdge-queue
variants all measured slower end-to-end), DVE reciprocal + multiply. The LAST
chunk instead broadcasts via a rank-1 bf16 matmul (ones[1,65]^T @ recip row)
on the then-idle PE, which is lower latency than the DRAM trip.

Scheduling notes (all trace-driven):
- RoPE cos/sin muls read the projection PSUM directly (no cast) and both run
  on DVE; the swap matmul REUSES the projection PSUM bank (WAR tracked by the
  tile framework). gpsimd only runs the cos/sin table DMAs (swdge queue).
- Rope and V-copy thunks are staggered one tile behind their projection
  thunks so a matmul whose DVE inputs are still in flight never heads the
  in-order PE queue (group 0's V copies stay adjacent: chunk 0 consumes them
  at the very next drain).
- The Act hwdge queue carries only x pieces 0/2: anything else on it delays
  exp issue (Act runs all 136 exps, ~122us, and is the #2 engine).
- wo + the last x piece load late on sync, after everything that gates the
  first scores.
- The last wo group streams each 128-col z slice out per-mi instead of one
  batched DMA.

Layout notes: scores are computed k-major ([key 128, head, q]) so PV contracts
keys on partitions. PSUM plan per chunk: scores 2 bufs x [128,2,512] (2 banks
each) + pv [65,2,512] (2 banks) + proj/wo-z 2 bufs x [128,512] (1 bank each) =
8 banks exactly. RoPE: deinterleaved pairs, SWAP matmul.

Perf history (HW exec): v1 DRAM-roundtrip-per-chunk 239.9us -> lead-in DMA
re-queueing 231.6 -> tail PE-broadcast + per-mi out DMA 229.6 -> rope
restructure + staggering + cs prefetch + diag-first 220.6-221.2us.
fp8 DoubleRow scores were tried and REVERTED: DR packs 2 contraction rows per
PE cell (helps only K>128; time is always 1 col/cycle), so with K=64 it
cannot beat bf16; measured equal-at-best, and sub-64-partition DR NaN'd.
"""

import sys

if "/opt/trn_rl_repo" not in sys.path:
    sys.path.insert(0, "/opt/trn_rl_repo")

import numpy as np
import ml_dtypes

import concourse.bass as bass
import concourse.tile as tile
from concourse import bacc, library_config, mybir

F32 = mybir.dt.float32
BF16 = mybir.dt.bfloat16
FP8 = mybir.dt.float8e4
DR = mybir.MatmulPerfMode.DoubleRow

B, S, D = 4, 2048, 768
H, KVH, HD = 12, 4, 64
HL, KVL = 6, 2          # local q heads / kv heads per core
NQC = 16                # query chunks of 128


def build_nc(phases=3):
    nc = bacc.Bacc(None, target_bir_lowering=False)

    xT = nc.dram_tensor("xT", [D, S], BF16, kind="ExternalInput")
    wq = nc.dram_tensor("wq", [D, HL * HD], BF16, kind="ExternalInput")
    wk = nc.dram_tensor("wk", [D, KVL * HD], BF16, kind="ExternalInput")
    wv = nc.dram_tensor("wv", [D, KVL * HD], BF16, kind="ExternalInput")
    wo = nc.dram_tensor("wo", [HL * HD, D], BF16, kind="ExternalInput")
    kcs = nc.dram_tensor("kcs", [64, S], BF16, kind="ExternalInput")  # [cos;sin]
    maskd = nc.dram_tensor("maskd", [128, 2, 384], BF16, kind="ExternalInput")
    out = nc.dram_tensor("out", [D, S], BF16, kind="ExternalOutput")
    rec_dram = nc.dram_tensor("rec_scratch", [NQC, 768], F32, kind="Internal")

    # swap matrix: exchanges 32-partition blocks 0<->1, 2<->3 (rope pairs)
    SW = np.zeros((128, 128), ml_dtypes.bfloat16)
    for blk in range(4):
        srcb = blk ^ 1
        for i in range(32):
            SW[blk * 32 + i, srcb * 32 + i] = 1.0
    sw_dram = nc.inline_tensor(SW, name="swconst")
    sign = np.zeros((128, 1), np.float32)
    for blk in range(4):
        sign[blk * 32:(blk + 1) * 32] = -1.0 if blk % 2 == 0 else 1.0
    sign_dram = nc.inline_tensor(sign, name="signconst")
    id_dram = nc.inline_tensor(np.eye(128, dtype=ml_dtypes.bfloat16),
                               name="idconst")

    def mm(out_ap, lhsT, rhs, start, stop, tile_position=None):
        nc.tensor.matmul(
            out_ap, lhsT, rhs,
            start=start, stop=stop,
            tile_position=tile_position,
            skip_group_check=True,
        )

    with tile.TileContext(nc) as tc:
        with tc.tile_pool(name="persist", bufs=1) as persist, \
             tc.tile_pool(name="p1", bufs=1) as p1, \
             tc.tile_pool(name="cs", bufs=2) as csp, \
             tc.tile_pool(name="tmp", bufs=2) as tmpp, \
             tc.tile_pool(name="rop", bufs=2) as ropp, \
             tc.tile_pool(name="expp", bufs=3) as expp, \
             tc.tile_pool(name="nrm", bufs=2) as nrm, \
             tc.tile_pool(name="at2p", bufs=6) as at2p, \
             tc.tile_pool(name="zp", bufs=3) as zp, \
             tc.tile_pool(name="psSC", bufs=2, space="PSUM") as psSC, \
             tc.tile_pool(name="psPV", bufs=1, space="PSUM") as psPV, \
             tc.tile_pool(name="psM", bufs=2, space="PSUM") as psM:

            qT = persist.tile([64, HL, S], BF16)
            kT = persist.tile([64, KVL, S], BF16)
            Vg = persist.tile([128, 16, 130], BF16)
            wo_sb = persist.tile([128, 3, D], BF16)
            mask_sb = persist.tile([128, 2, 384], BF16)
            sw_sb = persist.tile([128, 128], BF16)
            id_sb = persist.tile([128, 128], BF16)
            sign_sb = persist.tile([128, 1], F32)
            gpwarm = persist.tile([128, 1], BF16)
            ones_sb = persist.tile([1, 65], BF16)
            xT_sb = p1.tile([128, 6, S], BF16)
            wq_sb = p1.tile([128, 6, HL * HD], BF16)
            wk_sb = p1.tile([128, 6, KVL * HD], BF16)
            wv_sb = p1.tile([128, 6, KVL * HD], BF16)

            # warm the gpsimd library early (first real gp op is a rope
            # sin-mul; the lib load costs ~0.5us and sat on the critical
            # path in v1)
            nc.gpsimd.memset(gpwarm[:, :], 0.0)

            def cs_bcast_dma(dst, col0, width, row0):
                base = kcs[row0:row0 + 32, col0:col0 + width]
                ap = bass.AP(tensor=base.tensor, offset=base.offset,
                             ap=[[0, 4]] + list(base.ap))
                nc.gpsimd.dma_start(out=dst, in_=ap)

            def load_rowtiled(dst, srct, nrows, ncols, ntile):
                # dst [128, ntile, ncols] <- src [ntile*128, ncols], one DMA
                ap = bass.AP(tensor=srct, offset=0,
                             ap=[[ncols, 128], [128 * ncols, ntile],
                                 [1, ncols]])
                nc.sync.dma_start(out=dst, in_=ap)

            def xT_piece(qt, eng):
                ap = bass.AP(tensor=xT, offset=qt * 512,
                             ap=[[S, 128], [128 * S, 6], [1, 512]])
                eng.dma_start(out=xT_sb[:, :, qt * 512:(qt + 1) * 512],
                              in_=ap)

            # weights first on the sync queue: first matmuls gate on wk + x.
            # x pieces 0/2 ride the Act queue, 1/3 the sync queue so piece 1
            # (chunk-1 projection, PE-head at ~17us) is not stuck behind
            # piece 0's transfer on a single queue.
            load_rowtiled(wk_sb[:, :, :], wk, 768, KVL * HD, 6)
            xT_piece(1, nc.scalar)
            load_rowtiled(wq_sb[:, :, :], wq, 768, HL * HD, 6)
            load_rowtiled(wv_sb[:, :, :], wv, 768, KVL * HD, 6)
            cs0_cos = csp.tile([128, 512], BF16, tag="cos")
            cs0_sin = csp.tile([128, 512], BF16, tag="sin")
            cs_bcast_dma(cs0_cos[:, :], 0, 512, 0)
            cs_bcast_dma(cs0_sin[:, :], 0, 512, 32)
            xT_piece(2, nc.scalar)
            xT_piece(0, nc.sync)
            nc.sync.dma_start(out=sw_sb[:, :], in_=sw_dram[:, :])
            nc.sync.dma_start(out=sign_sb[:, :], in_=sign_dram[:, :])
            nc.sync.dma_start(out=id_sb[:, :], in_=id_dram[:, :])
            ap = bass.AP(tensor=wo, offset=0,
                         ap=[[D, 128], [128 * D, 3], [1, D]])
            nc.sync.dma_start(out=wo_sb[:, :, :], in_=ap)
            nc.sync.dma_start(out=mask_sb[:, :, :], in_=maskd[:, :, :])
            xT_piece(3, nc.sync)
            nc.vector.memset(Vg[:, :, 0:1], 1.0)
            nc.vector.memset(Vg[:, :, 65:66], 1.0)
            nc.vector.memset(ones_sb[:, :], 1.0)

            # preload the Act exp table before attention needs it; no
            # PE warmup matmuls (HAM trace shows full clock at startup)
            warm_sb = tmpp.tile([128, 512], F32, tag="warm")
            nc.scalar.activation(
                out=warm_sb[:, 0:128], in_=id_sb[:, 0:128],
                func=mybir.ActivationFunctionType.Exp, scale=0.01)

            pend = []   # deferred projection / wo thunks for PE slack

            def drain(n):
                for _ in range(min(n, len(pend))):
                    pend.pop(0)()

            # ---- projection thunks (chunk c of 512 cols: K, 3xQ, 4xV) ----
            def queue_proj(c, deep=False):
                cst = {}
                state = {"n": 0}

                def palloc():
                    # during the lead-in, borrow idle score-pool banks to
                    # double the rope pipeline depth
                    if deep and state["n"] % 2 == 1:
                        t = psSC.tile([128, 2, 512], F32, tag="sc")
                        state["n"] += 1
                        return t[:, 0, :]
                    state["n"] += 1
                    t = psM.tile([128, 512], F32, tag="pj")
                    return t[:, :]
                if c == 0:
                    cst["cos"], cst["sin"] = cs0_cos, cs0_sin
                else:
                    # issue the table loads NOW (not deferred): the swdge
                    # transfer takes a few us and was stalling the group's
                    # first rope
                    cos_t = csp.tile([128, 512], BF16, tag="cos")
                    sin_t = csp.tile([128, 512], BF16, tag="sin")
                    cs_bcast_dma(cos_t[:, :], c * 512, 512, 0)
                    cs_bcast_dma(sin_t[:, :], c * 512, 512, 32)
                    cst["cos"], cst["sin"] = cos_t, sin_t

                def mtile(m):
                    # m = 0: K pair tile; m = 1..3: Q pair tile m-1
                    st = {}

                    def t_proj_piece(d0):
                        # 2 accumulation matmuls per thunk: fine enough to
                        # interleave with attention without queue lumps
                        if d0 == 0:
                            pj_t = palloc()
                            st["pj"] = pj_t
                        pj = st["pj"]
                        for dt in (d0, d0 + 1):
                            if m == 0:
                                mm(pj, wk_sb[:, dt, :],
                                   xT_sb[:, dt, c * 512:(c + 1) * 512],
                                   start=(dt == 0), stop=(dt == 5))
                            else:
                                t = m - 1
                                mm(pj,
                                   wq_sb[:, dt, t * 128:(t + 1) * 128],
                                   xT_sb[:, dt, c * 512:(c + 1) * 512],
                                   start=(dt == 0), stop=(dt == 5))
                    def t_rope():
                        # cos/sin muls read the projection PSUM directly:
                        # no tcp cast, no gpsimd in the chain (its 1.1-1.7us
                        # sin-muls were the head-of-line pacer for the PE)
                        pj = st["pj"]
                        ta = tmpp.tile([128, 512], BF16, tag="ta")
                        nc.vector.tensor_mul(ta[:, :], pj, cst["cos"])
                        tb = tmpp.tile([128, 512], BF16, tag="tb")
                        nc.vector.tensor_mul(tb[:, :], pj, cst["sin"])
                        # the swap matmul overwrites the projection PSUM
                        # (WAR on the muls, tracked by the tile framework):
                        # halves psM pressure vs a fresh bank
                        sw_ps = pj
                        mm(sw_ps, sw_sb[:, :], tb[:, :],
                           start=True, stop=True)
                        ro = ropp.tile([128, 512], BF16, tag="ro")
                        nc.vector.scalar_tensor_tensor(
                            out=ro[:, :], in0=sw_ps,
                            scalar=sign_sb[:, 0:1], in1=ta[:, :],
                            op0=mybir.AluOpType.mult,
                            op1=mybir.AluOpType.add)
                        if m == 0:
                            nc.sync.dma_start(
                                out=kT[:, 0, c * 512:(c + 1) * 512],
                                in_=ro[0:64, :])
                            nc.sync.dma_start(
                                out=kT[:, 1, c * 512:(c + 1) * 512],
                                in_=ro[64:128, :])
                        else:
                            t = m - 1
                            nc.sync.dma_start(
                                out=qT[:, 2 * t, c * 512:(c + 1) * 512],
                                in_=ro[0:64, :])
                            nc.sync.dma_start(
                                out=qT[:, 2 * t + 1,
                                       c * 512:(c + 1) * 512],
                                in_=ro[64:128, :])
                    return ([lambda d0=d0: t_proj_piece(d0)
                             for d0 in (0, 2, 4)], t_rope)

                # stagger each rope one projection-tile behind its own
                # pieces so the swap matmul never heads the PE queue while
                # its DVE muls are still running
                mt = [mtile(m) for m in range(4)]
                pend.extend(mt[0][0])
                pend.extend(mt[1][0])
                pend.append(mt[0][1])
                pend.extend(mt[2][0])
                pend.append(mt[1][1])
                pend.extend(mt[3][0])
                pend.append(mt[2][1])
                pend.append(mt[3][1])

                def vtile(st_):
                    vs = {}

                    def t_v_half(d0):
                        if d0 == 0:
                            vp_t = psM.tile([128, 512], F32, tag="pj")
                            vs["vp"] = vp_t
                        vp = vs["vp"]
                        for dt in (d0, d0 + 1, d0 + 2):
                            mm(vp[:, 0:128],
                               xT_sb[:, dt, st_ * 128:(st_ + 1) * 128],
                               wv_sb[:, dt, :],
                               start=(dt == 0), stop=(dt == 5))
                        if d0 == 0:
                            return

                    def t_v():
                        vp = vs["vp"]
                        base = Vg[:, st_, 1:65]
                        dst = bass.AP(tensor=Vg.tensor, offset=base.offset,
                                      ap=[list(base.ap)[0], [65, 2],
                                          [1, 64]])
                        nc.vector.tensor_copy(dst, vp[:, 0:128])
                    return ([lambda: t_v_half(0), lambda: t_v_half(3)], t_v)

                # stagger each V copy one tile behind its matmuls, same
                # head-of-line reasoning as the rope stagger above. Group 0
                # keeps copies adjacent: chunk 0's PV reads Vg[0] at the
                # very next drain, so its copy cannot lag.
                vt = [vtile(st_) for st_ in range(4 * c, 4 * c + 4)]
                if c == 0:
                    for ps, cp in vt:
                        pend.extend(ps)
                        pend.append(cp)
                else:
                    pend.extend(vt[0][0])
                    pend.extend(vt[1][0])
                    pend.append(vt[0][1])
                    pend.extend(vt[2][0])
                    pend.append(vt[1][1])
                    pend.extend(vt[3][0])
                    pend.append(vt[2][1])
                    pend.append(vt[3][1])

            # ---- wo thunks (batch bi: chunks 3bi .. 3bi+nch-1) ----
            at2s = [None] * 6

            def queue_wo(bi, nch):
                attnT2 = at2s[bi]
                ncols = nch * 128
                zs = {}

                def t_zalloc():
                    z_sb = zp.tile([128, 6, 384], BF16, tag="zsb")
                    zs["sb"] = z_sb
                pend.append(t_zalloc)
                for mi in range(6):
                    zt = psM.tile([128, 512], F32, tag="pj")
                    for p in range(3):
                        pend.append(
                            lambda zt=zt, mi=mi, p=p:
                            mm(zt[:, 0:ncols],
                               wo_sb[:, p, mi * 128:(mi + 1) * 128],
                               attnT2[:, p, 0:ncols],
                               start=(p == 0), stop=(p == 2)))

                    def t_copy(zt=zt, mi=mi):
                        nc.vector.tensor_copy(zs["sb"][:, mi, 0:ncols],
                                              zt[:, 0:ncols])
                        if nch == 1:
                            # tail group: stream each mi out as it lands
                            # instead of one big DMA after all six copies
                            dst = bass.AP(
                                tensor=out,
                                offset=mi * 128 * S + bi * 384,
                                ap=[[S, 128], [1, ncols]])
                            nc.sync.dma_start(out=dst,
                                              in_=zs["sb"][:, mi, 0:ncols])
                    pend.append(t_copy)

                if nch > 1:
                    def t_out():
                        dst = bass.AP(
                            tensor=out, offset=bi * 384,
                            ap=[[S, 128], [128 * S, 6], [1, ncols]])
                        src_ap = zs["sb"][:, :, 0:ncols]
                        nc.sync.dma_start(out=dst, in_=src_ap)
                    pend.append(t_out)

            # ---- fused main loop ----
            queue_proj(0, deep=True)
            drain(16)                 # K + Q m-tiles up front; V deferred
            queue_proj(1)

            for j2 in range(NQC):
                if j2 == 4:
                    queue_proj(2)
                elif j2 == 8:
                    queue_proj(3)
                elif j2 == 12:
                    for bi in range(4):
                        queue_wo(bi, 3)
                elif j2 == 15:
                    queue_wo(4, 3)
                pv = psPV.tile([65, 2, 512], F32)
                prev = None
                # PV accumulation over kt is commutative; for the LAST chunk
                # process the diagonal tile FIRST so its exp + mask multiply
                # leave the serial tail chain
                kts = list(range(j2 + 1))
                if j2 == NQC - 1:
                    kts = [j2] + kts[:-1]
                for ki in range(j2 + 2):
                    if prev is not None:
                        for g in range(2):
                            mm(pv[0:65, g, 0:384],
                               Vg[:, prev[1], g * 65:g * 65 + 65],
                               prev[0][:, g, :],
                               start=prev[2], stop=prev[3])
                    if ki > j2:
                        break
                    kt = kts[ki]
                    sc = psSC.tile([128, 2, 512], F32, tag="sc")
                    for g in range(2):
                        mm(sc[:, g, 0:384],
                           kT[:, g, kt * 128:(kt + 1) * 128],
                           qT[:, 3 * g:3 * g + 3,
                              j2 * 128:(j2 + 1) * 128],
                           start=True, stop=True)
                    drain(3)
                    expT = expp.tile([128, 2, 384], BF16, tag="expT")
                    nc.scalar.activation(
                        out=expT[:, :, :],
                        in_=sc[:, :, 0:384],
                        func=mybir.ActivationFunctionType.Exp,
                        scale=0.125,
                    )
                    if kt == j2:
                        nc.vector.tensor_mul(
                            expT[:, :, :], expT[:, :, :], mask_sb[:, :, :])
                    prev = (expT, kt, ki == 0, ki == j2)
                if j2 in (3, 7, 11):
                    drain(len(pend))  # projection must land before next group

                # ---- normalization: denom is PV row 0 (ones col at 0) ----
                pvf = nrm.tile([65, 2, 384], F32, tag="pvf")
                nc.vector.tensor_copy(pvf[:, :, :], pv[0:65, :, 0:384])
                attn = nrm.tile([65, 2, 384], BF16, tag="attn")
                if j2 == NQC - 1:
                    # tail chunk: PE is idle here, so broadcast the
                    # reciprocal row across partitions with a rank-1 fp32
                    # matmul (ones[1,65]^T @ recd[1,768]) instead of the
                    # higher-latency DRAM round trip
                    recd = nrm.tile([1, 2, 384], F32, tag="recd")
                    nc.vector.reciprocal_approx_fast(recd[:, :, :],
                                                     pvf[0:1, :, :])
                    recd_bf = nrm.tile([1, 2, 384], BF16, tag="recdbf")
                    nc.vector.tensor_copy(recd_bf[:, :, :], recd[:, :, :])
                    bc = psSC.tile([128, 2, 512], F32, tag="sc")
                    for g in range(2):
                        mm(bc[0:65, g, 0:384], ones_sb[0:1, :],
                           recd_bf[0:1, g, :], start=True, stop=True)
                    nc.vector.tensor_mul(attn[:, :, :], pvf[:, :, :],
                                         bc[0:65, :, 0:384])
                else:
                    # denom row -> DRAM -> broadcast read (DRAM replicated
                    # reads are fast; SBUF single-partition replication and
                    # gpsimd partition_broadcast both measured slower)
                    nc.sync.dma_start(out=rec_dram[j2, :], in_=pvf[0:1, :, :])
                    denb = nrm.tile([65, 2, 384], F32, tag="denb")
                    rb_src = bass.AP(tensor=rec_dram, offset=j2 * 768,
                                     ap=[[0, 65], [1, 768]])
                    nc.sync.dma_start(out=denb[:, :, :], in_=rb_src)
                    recb = nrm.tile([65, 2, 384], F32, tag="recb")
                    nc.vector.reciprocal_approx_fast(recb[:, :, :],
                                                     denb[:, :, :])
                    nc.vector.tensor_mul(attn[:, :, :], pvf[:, :, :],
                                         recb[:, :, :])
                if j2 % 3 == 0:
                    at2_tile = at2p.tile([128, 3, 384], BF16, tag="at2")
                    at2s[j2 // 3] = at2_tile
                attnT2 = at2s[j2 // 3]
                cslot = (j2 % 3) * 128
                for par in range(2):
                    # heads h with h%2==par live at free offsets
                    # 128*(par + 2k) in attn rows 1:65: uniform stride 256
                    sbase = attn[1:65, 0, par * 128:par * 128 + 128]
                    s_ap = bass.AP(tensor=sbase.tensor, offset=sbase.offset,
                                   ap=[list(sbase.ap)[0], [256, 3],
                                       [1, 128]])
                    dbase = attnT2[64 * par:64 * par + 64, 0,
                                   cslot:cslot + 128]
                    d_ap = bass.AP(tensor=dbase.tensor, offset=dbase.offset,
                                   ap=[list(dbase.ap)[0], [384, 3],
                                       [1, 128]])
                    # tail chunk: run the two transposes on separate queues
                    eng = nc.scalar if (j2 == NQC - 1 and par == 1) \
                        else nc.sync
                    eng.dma_start(out=d_ap, in_=s_ap)

            drain(len(pend))
            queue_wo(5, 1)
            drain(len(pend))

    nc.compile()
    return nc


# ---------------------------------------------------------------------------
# host side
# ---------------------------------------------------------------------------

def _permute_cols(w, nheads):
    """Deinterleave rope pairs within each head: even dims then odd dims."""
    cols = []
    for h in range(nheads):
        blk = w[:, h * HD:(h + 1) * HD]
        cols.append(blk[:, 0::2])
        cols.append(blk[:, 1::2])
    return np.ascontiguousarray(np.concatenate(cols, axis=1))


def make_in_maps(x, wq, wk, wv, wo, freqs_cos, freqs_sin):
    bf = ml_dtypes.bfloat16
    wq = np.asarray(wq, np.float32)
    wk = np.asarray(wk, np.float32)
    wv = np.asarray(wv, np.float32)
    wo = np.asarray(wo, np.float32)

    cosT = np.ascontiguousarray(np.asarray(freqs_cos, np.float32).T)  # [32, S]
    sinT = np.ascontiguousarray(np.asarray(freqs_sin, np.float32).T)
    kcs = np.ascontiguousarray(np.concatenate([cosT, sinT], axis=0))  # [64, S]

    # constant diagonal mask [128, 2, 384]: m[k, :, i*128 + q] = (k <= q)
    tri = (np.arange(128)[:, None] <= np.arange(128)[None, :])
    maskd = np.broadcast_to(tri[:, None, None, :],
                            (128, 2, 3, 128)).reshape(128, 2, 384)
    maskd = np.ascontiguousarray(maskd.astype(np.float32))

    in_maps = []
    for core in range(8):
        b, half = core // 2, core % 2
        xT = np.ascontiguousarray(np.asarray(x[b], np.float32).T)
        qh = slice(half * HL * HD, (half + 1) * HL * HD)
        kh = slice(half * KVL * HD, (half + 1) * KVL * HD)
        in_maps.append({
            "xT": xT.astype(bf),
            "wq": _permute_cols(wq[:, qh], HL).astype(bf),
            "wk": _permute_cols(wk[:, kh], KVL).astype(bf),
            "wv": np.ascontiguousarray(wv[:, kh]).astype(bf),
            "wo": np.ascontiguousarray(wo[qh, :]).astype(bf),
            "kcs": kcs.astype(bf),
            "maskd": maskd.astype(bf),
        })
    return in_maps


_NC_CACHE = {}


def kernel(x, wq, wk, wv, wo, freqs_cos, freqs_sin, mask_attention,
           start_pos=0, inference=0, **_ignored):
    from concourse.bass_utils import run_bass_kernel_spmd

    in_maps = make_in_maps(np.asarray(x, np.float32), wq, wk, wv, wo,
                           freqs_cos, freqs_sin)
    if "nc" not in _NC_CACHE:
        _NC_CACHE["nc"] = build_nc()
    nc = _NC_CACHE["nc"]
    res = run_bass_kernel_spmd(nc, in_maps, core_ids=list(range(8)))
    outs = res.results
    out_full = np.zeros((B, S, D), np.float32)
    for b in range(B):
        z0 = np.asarray(outs[2 * b]["out"], np.float32)
        z1 = np.asarray(outs[2 * b + 1]["out"], np.float32)
        out_full[b] = (z0 + z1).T
    return out_full


# revision 60
# speedup vs baseline: 1.0439x; 1.0439x over previous
"""Trainium2 Bass kernel for GQA attention (B=4, S=2048, D=768, H=12, KVH=4, HD=64).

Sharding: core = (batch, head-half). Each core computes 6 query heads and its
2 KV heads over the full sequence for one batch; the wo projection produces a
partial [768, 2048] output (6 of 12 head contributions) and the host adds the
two halves per batch. No collectives; K/V projection work is fully partitioned
across cores (no duplication).

Attention uses exact causal tiling with 128-row query chunks: chunk j2 attends
key tiles 0..j2, so no fully-masked tiles are ever computed; only the diagonal
tile needs a (single, constant) triangular mask, applied post-exp as a DVE
multiply. For the last chunk the diagonal tile is processed FIRST (the PV
accumulation is commutative) so that multiply leaves the serial tail chain.

The softmax denominator is folded into the PV matmul: V tiles carry an
all-ones column at position 0, so the PV output row 0 accumulates sum(exp) for
free and lands on PSUM partition 0. Normalization: write row 0 to DRAM, read
it back replicated across 65 partitions (DRAM replicated reads are fast; SBUF
single-partition replication, gpsimd partition_broadcast, and swdge-queue
variants all measured slower end-to-end), DVE reciprocal + multiply. The LAST
chunk instead broadcasts via a rank-1 bf16 matmul (ones[1,65]^T @ recip row)
on the then-idle PE, which is lower latency than the DRAM trip.

Scheduling notes (all trace-driven):
- RoPE cos/sin muls read the projection PSUM directly (no cast) and both run
  on DVE; the swap matmul REUSES the projection PSUM bank (WAR tracked by the
  tile framework). gpsimd only runs the cos/sin table DMAs (swdge queue).
- Rope and V-copy thunks are staggered one tile behind their projection
  thunks so a matmul whose DVE inputs are still in flight never heads the
  in-order PE queue (group 0's V copies stay adjacent: chunk 0 consumes them
  at the very next drain).
- The Act hwdge queue carries only x pieces 0/2: anything else on it delays
  exp issue (Act runs all 136 exps, ~122us, and is the #2 engine).
- wo + the last x piece load late on sync, after everything that gates the
  first scores.
- The last wo group streams each 128-col z slice out per-mi instead of one
  batched DMA.

Layout notes: scores are computed k-major ([key 128, head, q]) so PV contracts
keys on partitions. PSUM plan per chunk: scores 2 bufs x [128,2,512] (2 banks
each) + pv [65,2,512] (2 banks) + proj/wo-z 2 bufs x [128,512] (1 bank each) =
8 banks exactly. RoPE: deinterleaved pairs, SWAP matmul.

Perf history (HW exec): v1 DRAM-roundtrip-per-chunk 239.9us -> lead-in DMA
re-queueing 231.6 -> tail PE-broadcast + per-mi out DMA 229.6 -> rope
restructure + staggering + cs prefetch + diag-first 220.6-221.2us.
fp8 DoubleRow scores were tried and REVERTED: DR packs 2 contraction rows per
PE cell (helps only K>128; time is always 1 col/cycle), so with K=64 it
cannot beat bf16; measured equal-at-best, and sub-64-partition DR NaN'd.
"""

import sys

if "/opt/trn_rl_repo" not in sys.path:
    sys.path.insert(0, "/opt/trn_rl_repo")

import numpy as np
import ml_dtypes

import concourse.bass as bass
import concourse.tile as tile
from concourse import bacc, library_config, mybir

F32 = mybir.dt.float32
BF16 = mybir.dt.bfloat16
FP8 = mybir.dt.float8e4
DR = mybir.MatmulPerfMode.DoubleRow

B, S, D = 4, 2048, 768
H, KVH, HD = 12, 4, 64
HL, KVL = 6, 2          # local q heads / kv heads per core
NQC = 16                # query chunks of 128


def build_nc(phases=3):
    nc = bacc.Bacc(None, target_bir_lowering=False)

    xT = nc.dram_tensor("xT", [D, S], BF16, kind="ExternalInput")
    wq = nc.dram_tensor("wq", [D, HL * HD], BF16, kind="ExternalInput")
    wk = nc.dram_tensor("wk", [D, KVL * HD], BF16, kind="ExternalInput")
    wv = nc.dram_tensor("wv", [D, KVL * HD], BF16, kind="ExternalInput")
    wo = nc.dram_tensor("wo", [HL * HD, D], BF16, kind="ExternalInput")
    kcs = nc.dram_tensor("kcs", [64, S], BF16, kind="ExternalInput")  # [cos;sin]
    maskd = nc.dram_tensor("maskd", [128, 2, 384], BF16, kind="ExternalInput")
    out = nc.dram_tensor("out", [D, S], BF16, kind="ExternalOutput")
    rec_dram = nc.dram_tensor("rec_scratch", [NQC, 768], F32, kind="Internal")

    # swap matrix: exchanges 32-partition blocks 0<->1, 2<->3 (rope pairs)
    SW = np.zeros((128, 128), ml_dtypes.bfloat16)
    for blk in range(4):
        srcb = blk ^ 1
        for i in range(32):
            SW[blk * 32 + i, srcb * 32 + i] = 1.0
    sw_dram = nc.inline_tensor(SW, name="swconst")
    sign = np.zeros((128, 1), np.float32)
    for blk in range(4):
        sign[blk * 32:(blk + 1) * 32] = -1.0 if blk % 2 == 0 else 1.0
    sign_dram = nc.inline_tensor(sign, name="signconst")
    id_dram = nc.inline_tensor(np.eye(128, dtype=ml_dtypes.bfloat16),
                               name="idconst")

    def mm(out_ap, lhsT, rhs, start, stop, tile_position=None):
        nc.tensor.matmul(
            out_ap, lhsT, rhs,
            start=start, stop=stop,
            tile_position=tile_position,
            skip_group_check=True,
        )

    with tile.TileContext(nc) as tc:
        with tc.tile_pool(name="persist", bufs=1) as persist, \
             tc.tile_pool(name="p1", bufs=1) as p1, \
             tc.tile_pool(name="cs", bufs=2) as csp, \
             tc.tile_pool(name="tmp", bufs=2) as tmpp, \
             tc.tile_pool(name="rop", bufs=2) as ropp, \
             tc.tile_pool(name="expp", bufs=3) as expp, \
             tc.tile_pool(name="nrm", bufs=2) as nrm, \
             tc.tile_pool(name="at2p", bufs=6) as at2p, \
             tc.tile_pool(name="zp", bufs=3) as zp, \
             tc.tile_pool(name="psSC", bufs=2, space="PSUM") as psSC, \
             tc.tile_pool(name="psPV", bufs=1, space="PSUM") as psPV, \
             tc.tile_pool(name="psM", bufs=2, space="PSUM") as psM:

            qT = persist.tile([64, HL, S], BF16)
            kT = persist.tile([64, KVL, S], BF16)
            Vg = persist.tile([128, 16, 130], BF16)
            wo_sb = persist.tile([128, 3, D], BF16)
            mask_sb = persist.tile([128, 2, 384], BF16)
            sw_sb = persist.tile([128, 128], BF16)
            id_sb = persist.tile([128, 128], BF16)
            sign_sb = persist.tile([128, 1], F32)
            gpwarm = persist.tile([128, 1], BF16)
            ones_sb = persist.tile([1, 65], BF16)
            xT_sb = p1.tile([128, 6, S], BF16)
            wq_sb = p1.tile([128, 6, HL * HD], BF16)
            wk_sb = p1.tile([128, 6, KVL * HD], BF16)
            wv_sb = p1.tile([128, 6, KVL * HD], BF16)

            # warm the gpsimd library early (first real gp op is a rope
            # sin-mul; the lib load costs ~0.5us and sat on the critical
            # path in v1)
            nc.gpsimd.memset(gpwarm[:, :], 0.0)

            def cs_bcast_dma(dst, col0, width, row0):
                base = kcs[row0:row0 + 32, col0:col0 + width]
                ap = bass.AP(tensor=base.tensor, offset=base.offset,
                             ap=[[0, 4]] + list(base.ap))
                nc.gpsimd.dma_start(out=dst, in_=ap)

            def load_rowtiled(dst, srct, nrows, ncols, ntile):
                # dst [128, ntile, ncols] <- src [ntile*128, ncols], one DMA
                ap = bass.AP(tensor=srct, offset=0,
                             ap=[[ncols, 128], [128 * ncols, ntile],
                                 [1, ncols]])
                nc.sync.dma_start(out=dst, in_=ap)

            def xT_piece(qt, eng):
                ap = bass.AP(tensor=xT, offset=qt * 512,
                             ap=[[S, 128], [128 * S, 6], [1, 512]])
                eng.dma_start(out=xT_sb[:, :, qt * 512:(qt + 1) * 512],
                              in_=ap)

            # weights first on the sync queue: first matmuls gate on wk + x.
            # x pieces 0/2 ride the Act queue, 1/3 the sync queue so piece 1
            # (chunk-1 projection, PE-head at ~17us) is not stuck behind
            # piece 0's transfer on a single queue.
            load_rowtiled(wk_sb[:, :, :], wk, 768, KVL * HD, 6)
            xT_piece(0, nc.scalar)
            load_rowtiled(wq_sb[:, :, :], wq, 768, HL * HD, 6)
            load_rowtiled(wv_sb[:, :, :], wv, 768, KVL * HD, 6)
            cs0_cos = csp.tile([128, 512], BF16, tag="cos")
            cs0_sin = csp.tile([128, 512], BF16, tag="sin")
            cs_bcast_dma(cs0_cos[:, :], 0, 512, 0)
            cs_bcast_dma(cs0_sin[:, :], 0, 512, 32)
            xT_piece(2, nc.scalar)
            xT_piece(1, nc.sync)
            nc.sync.dma_start(out=sw_sb[:, :], in_=sw_dram[:, :])
            nc.sync.dma_start(out=sign_sb[:, :], in_=sign_dram[:, :])
            nc.sync.dma_start(out=id_sb[:, :], in_=id_dram[:, :])
            ap = bass.AP(tensor=wo, offset=0,
                         ap=[[D, 128], [128 * D, 3], [1, D]])
            nc.sync.dma_start(out=wo_sb[:, :, :], in_=ap)
            nc.sync.dma_start(out=mask_sb[:, :, :], in_=maskd[:, :, :])
            xT_piece(3, nc.sync)
            nc.vector.memset(Vg[:, :, 0:1], 1.0)
            nc.vector.memset(Vg[:, :, 65:66], 1.0)
            nc.vector.memset(ones_sb[:, :], 1.0)

            # preload the Act exp table before attention needs it; no
            # PE warmup matmuls (HAM trace shows full clock at startup)
            warm_sb = tmpp.tile([128, 512], F32, tag="warm")
            nc.scalar.activation(
                out=warm_sb[:, 0:128], in_=id_sb[:, 0:128],
                func=mybir.ActivationFunctionType.Exp, scale=0.01)

            pend = []   # deferred projection / wo thunks for PE slack

            def drain(n):
                for _ in range(min(n, len(pend))):
                    pend.pop(0)()

            # ---- projection thunks (chunk c of 512 cols: K, 3xQ, 4xV) ----
            def queue_proj(c, deep=False):
                cst = {}
                state = {"n": 0}

                def palloc():
                    # during the lead-in, borrow idle score-pool banks to
                    # double the rope pipeline depth
                    if deep and state["n"] % 2 == 1:
                        t = psSC.tile([128, 2, 512], F32, tag="sc")
                        state["n"] += 1
                        return t[:, 0, :]
                    state["n"] += 1
                    t = psM.tile([128, 512], F32, tag="pj")
                    return t[:, :]
                if c == 0:
                    cst["cos"], cst["sin"] = cs0_cos, cs0_sin
                else:
                    # issue the table loads NOW (not deferred): the swdge
                    # transfer takes a few us and was stalling the group's
                    # first rope
                    cos_t = csp.tile([128, 512], BF16, tag="cos")
                    sin_t = csp.tile([128, 512], BF16, tag="sin")
                    cs_bcast_dma(cos_t[:, :], c * 512, 512, 0)
                    cs_bcast_dma(sin_t[:, :], c * 512, 512, 32)
                    cst["cos"], cst["sin"] = cos_t, sin_t

                def mtile(m):
                    # m = 0: K pair tile; m = 1..3: Q pair tile m-1
                    st = {}

                    def t_proj_piece(d0):
                        # 2 accumulation matmuls per thunk: fine enough to
                        # interleave with attention without queue lumps
                        if d0 == 0:
                            pj_t = palloc()
                            st["pj"] = pj_t
                        pj = st["pj"]
                        for dt in (d0, d0 + 1):
                            if m == 0:
                                mm(pj, wk_sb[:, dt, :],
                                   xT_sb[:, dt, c * 512:(c + 1) * 512],
                                   start=(dt == 0), stop=(dt == 5))
                            else:
                                t = m - 1
                                mm(pj,
                                   wq_sb[:, dt, t * 128:(t + 1) * 128],
                                   xT_sb[:, dt, c * 512:(c + 1) * 512],
                                   start=(dt == 0), stop=(dt == 5))
                    def t_rope():
                        # cos/sin muls read the projection PSUM directly:
                        # no tcp cast, no gpsimd in the chain (its 1.1-1.7us
                        # sin-muls were the head-of-line pacer for the PE)
                        pj = st["pj"]
                        ta = tmpp.tile([128, 512], BF16, tag="ta")
                        nc.vector.tensor_mul(ta[:, :], pj, cst["cos"])
                        tb = tmpp.tile([128, 512], BF16, tag="tb")
                        nc.vector.tensor_mul(tb[:, :], pj, cst["sin"])
                        # the swap matmul overwrites the projection PSUM
                        # (WAR on the muls, tracked by the tile framework):
                        # halves psM pressure vs a fresh bank
                        sw_ps = pj
                        mm(sw_ps, sw_sb[:, :], tb[:, :],
                           start=True, stop=True)
                        ro = ropp.tile([128, 512], BF16, tag="ro")
                        nc.vector.scalar_tensor_tensor(
                            out=ro[:, :], in0=sw_ps,
                            scalar=sign_sb[:, 0:1], in1=ta[:, :],
                            op0=mybir.AluOpType.mult,
                            op1=mybir.AluOpType.add)
                        if m == 0:
                            nc.sync.dma_start(
                                out=kT[:, 0, c * 512:(c + 1) * 512],
                                in_=ro[0:64, :])
                            nc.sync.dma_start(
                                out=kT[:, 1, c * 512:(c + 1) * 512],
                                in_=ro[64:128, :])
                        else:
                            t = m - 1
                            nc.sync.dma_start(
                                out=qT[:, 2 * t, c * 512:(c + 1) * 512],
                                in_=ro[0:64, :])
                            nc.sync.dma_start(
                                out=qT[:, 2 * t + 1,
                                       c * 512:(c + 1) * 512],
                                in_=ro[64:128, :])
                    return ([lambda d0=d0: t_proj_piece(d0)
                             for d0 in (0, 2, 4)], t_rope)

                # stagger each rope one projection-tile behind its own
                # pieces so the swap matmul never heads the PE queue while
                # its DVE muls are still running
                mt = [mtile(m) for m in range(4)]
                pend.extend(mt[0][0])
                pend.extend(mt[1][0])
                pend.append(mt[0][1])
                pend.extend(mt[2][0])
                pend.append(mt[1][1])
                pend.extend(mt[3][0])
                pend.append(mt[2][1])
                pend.append(mt[3][1])

                def vtile(st_):
                    vs = {}

                    def t_v_half(d0):
                        if d0 == 0:
                            vp_t = psM.tile([128, 512], F32, tag="pj")
                            vs["vp"] = vp_t
                        vp = vs["vp"]
                        for dt in (d0, d0 + 1, d0 + 2):
                            mm(vp[:, 0:128],
                               xT_sb[:, dt, st_ * 128:(st_ + 1) * 128],
                               wv_sb[:, dt, :],
                               start=(dt == 0), stop=(dt == 5))
                        if d0 == 0:
                            return

                    def t_v():
                        vp = vs["vp"]
                        base = Vg[:, st_, 1:65]
                        dst = bass.AP(tensor=Vg.tensor, offset=base.offset,
                                      ap=[list(base.ap)[0], [65, 2],
                                          [1, 64]])
                        nc.vector.tensor_copy(dst, vp[:, 0:128])
                    return ([lambda: t_v_half(0), lambda: t_v_half(3)], t_v)

                # stagger each V copy one tile behind its matmuls, same
                # head-of-line reasoning as the rope stagger above. Group 0
                # keeps copies adjacent: chunk 0's PV reads Vg[0] at the
                # very next drain, so its copy cannot lag.
                vt = [vtile(st_) for st_ in range(4 * c, 4 * c + 4)]
                if c == 0:
                    for ps, cp in vt:
                        pend.extend(ps)
                        pend.append(cp)
                else:
                    pend.extend(vt[0][0])
                    pend.extend(vt[1][0])
                    pend.append(vt[0][1])
                    pend.extend(vt[2][0])
                    pend.append(vt[1][1])
                    pend.extend(vt[3][0])
                    pend.append(vt[2][1])
                    pend.append(vt[3][1])

            # ---- wo thunks (batch bi: chunks 3bi .. 3bi+nch-1) ----
            at2s = [None] * 6

            def queue_wo(bi, nch):
                attnT2 = at2s[bi]
                ncols = nch * 128
                zs = {}

                def t_zalloc():
                    z_sb = zp.tile([128, 6, 384], BF16, tag="zsb")
                    zs["sb"] = z_sb
                pend.append(t_zalloc)
                for mi in range(6):
                    zt = psM.tile([128, 512], F32, tag="pj")
                    for p in range(3):
                        pend.append(
                            lambda zt=zt, mi=mi, p=p:
                            mm(zt[:, 0:ncols],
                               wo_sb[:, p, mi * 128:(mi + 1) * 128],
                               attnT2[:, p, 0:ncols],
                               start=(p == 0), stop=(p == 2)))

                    def t_copy(zt=zt, mi=mi):
                        nc.vector.tensor_copy(zs["sb"][:, mi, 0:ncols],
                                              zt[:, 0:ncols])
                        if nch == 1:
                            # tail group: stream each mi out as it lands
                            # instead of one big DMA after all six copies
                            dst = bass.AP(
                                tensor=out,
                                offset=mi * 128 * S + bi * 384,
                                ap=[[S, 128], [1, ncols]])
                            nc.sync.dma_start(out=dst,
                                              in_=zs["sb"][:, mi, 0:ncols])
                    pend.append(t_copy)

                if nch > 1:
                    def t_out():
                        dst = bass.AP(
                            tensor=out, offset=bi * 384,
                            ap=[[S, 128], [128 * S, 6], [1, ncols]])
                        src_ap = zs["sb"][:, :, 0:ncols]
                        nc.sync.dma_start(out=dst, in_=src_ap)
                    pend.append(t_out)

            # ---- fused main loop ----
            queue_proj(0, deep=True)
            drain(16)                 # K + Q m-tiles up front; V deferred
            queue_proj(1)

            for j2 in range(NQC):
                if j2 == 4:
                    queue_proj(2)
                elif j2 == 8:
                    queue_proj(3)
                elif j2 == 12:
                    for bi in range(4):
                        queue_wo(bi, 3)
                elif j2 == 15:
                    queue_wo(4, 3)
                pv = psPV.tile([65, 2, 512], F32)
                prev = None
                # PV accumulation over kt is commutative; for the LAST chunk
                # process the diagonal tile FIRST so its exp + mask multiply
                # leave the serial tail chain
                kts = list(range(j2 + 1))
                if j2 == NQC - 1:
                    kts = [j2] + kts[:-1]
                for ki in range(j2 + 2):
                    if prev is not None:
                        for g in range(2):
                            mm(pv[0:65, g, 0:384],
                               Vg[:, prev[1], g * 65:g * 65 + 65],
                               prev[0][:, g, :],
                               start=prev[2], stop=prev[3])
                    if ki > j2:
                        break
                    kt = kts[ki]
                    sc = psSC.tile([128, 2, 512], F32, tag="sc")
                    for g in range(2):
                        mm(sc[:, g, 0:384],
                           kT[:, g, kt * 128:(kt + 1) * 128],
                           qT[:, 3 * g:3 * g + 3,
                              j2 * 128:(j2 + 1) * 128],
                           start=True, stop=True)
                    drain(3)
                    expT = expp.tile([128, 2, 384], BF16, tag="expT")
                    nc.scalar.activation(
                        out=expT[:, :, :],
                        in_=sc[:, :, 0:384],
                        func=mybir.ActivationFunctionType.Exp,
                        scale=0.125,
                    )
                    if kt == j2:
                        nc.vector.tensor_mul(
                            expT[:, :, :], expT[:, :, :], mask_sb[:, :, :])
                    prev = (expT, kt, ki == 0, ki == j2)
                if j2 in (3, 7, 11):
                    drain(len(pend))  # projection must land before next group

                # ---- normalization: denom is PV row 0 (ones col at 0) ----
                pvf = nrm.tile([65, 2, 384], F32, tag="pvf")
                nc.vector.tensor_copy(pvf[:, :, :], pv[0:65, :, 0:384])
                attn = nrm.tile([65, 2, 384], BF16, tag="attn")
                if j2 == NQC - 1:
                    # tail chunk: PE is idle here, so broadcast the
                    # reciprocal row across partitions with a rank-1 fp32
                    # matmul (ones[1,65]^T @ recd[1,768]) instead of the
                    # higher-latency DRAM round trip
                    recd = nrm.tile([1, 2, 384], F32, tag="recd")
                    nc.vector.reciprocal_approx_fast(recd[:, :, :],
                                                     pvf[0:1, :, :])
                    recd_bf = nrm.tile([1, 2, 384], BF16, tag="recdbf")
                    nc.vector.tensor_copy(recd_bf[:, :, :], recd[:, :, :])
                    bc = psSC.tile([128, 2, 512], F32, tag="sc")
                    for g in range(2):
                        mm(bc[0:65, g, 0:384], ones_sb[0:1, :],
                           recd_bf[0:1, g, :], start=True, stop=True)
                    nc.vector.tensor_mul(attn[:, :, :], pvf[:, :, :],
                                         bc[0:65, :, 0:384])
                else:
                    # denom row -> DRAM -> broadcast read (DRAM replicated
                    # reads are fast; SBUF single-partition replication and
                    # gpsimd partition_broadcast both measured slower)
                    nc.sync.dma_start(out=rec_dram[j2, :], in_=pvf[0:1, :, :])
                    denb = nrm.tile([65, 2, 384], F32, tag="denb")
                    rb_src = bass.AP(tensor=rec_dram, offset=j2 * 768,
                                     ap=[[0, 65], [1, 768]])
                    nc.sync.dma_start(out=denb[:, :, :], in_=rb_src)
                    recb = nrm.tile([65, 2, 384], F32, tag="recb")
                    nc.vector.reciprocal_approx_fast(recb[:, :, :],
                                                     denb[:, :, :])
                    nc.vector.tensor_mul(attn[:, :, :], pvf[:, :, :],
                                         recb[:, :, :])
                if j2 % 3 == 0:
                    at2_tile = at2p.tile([128, 3, 384], BF16, tag="at2")
                    at2s[j2 // 3] = at2_tile
                attnT2 = at2s[j2 // 3]
                cslot = (j2 % 3) * 128
                if j2 == NQC - 1:
                    # tail chunk: per-(plane, parity) transposes, plane-major
                    # across two queues, so wo(5)'s p=0 accumulation starts
                    # after the first pair instead of after everything
                    for p in range(3):
                        for par in range(2):
                            h = 2 * p + par
                            src = attn[1:65, h // 3,
                                       (h % 3) * 128:(h % 3) * 128 + 128]
                            dst = attnT2[64 * par:64 * par + 64, p,
                                         cslot:cslot + 128]
                            eng = nc.scalar if par == 1 else nc.sync
                            eng.dma_start(out=dst, in_=src)
                else:
                    for par in range(2):
                        # heads h with h%2==par live at free offsets
                        # 128*(par + 2k) in attn rows 1:65: stride 256
                        sbase = attn[1:65, 0, par * 128:par * 128 + 128]
                        s_ap = bass.AP(tensor=sbase.tensor,
                                       offset=sbase.offset,
                                       ap=[list(sbase.ap)[0], [256, 3],
                                           [1, 128]])
                        dbase = attnT2[64 * par:64 * par + 64, 0,
                                       cslot:cslot + 128]
                        d_ap = bass.AP(tensor=dbase.tensor,
                                       offset=dbase.offset,
                                       ap=[list(dbase.ap)[0], [384, 3],
                                           [1, 128]])
                        nc.sync.dma_start(out=d_ap, in_=s_ap)

            drain(len(pend))
            queue_wo(5, 1)
            drain(len(pend))

    nc.compile()
    return nc


# ---------------------------------------------------------------------------
# host side
# ---------------------------------------------------------------------------

def _permute_cols(w, nheads):
    """Deinterleave rope pairs within each head: even dims then odd dims."""
    cols = []
    for h in range(nheads):
        blk = w[:, h * HD:(h + 1) * HD]
        cols.append(blk[:, 0::2])
        cols.append(blk[:, 1::2])
    return np.ascontiguousarray(np.concatenate(cols, axis=1))


def make_in_maps(x, wq, wk, wv, wo, freqs_cos, freqs_sin):
    bf = ml_dtypes.bfloat16
    wq = np.asarray(wq, np.float32)
    wk = np.asarray(wk, np.float32)
    wv = np.asarray(wv, np.float32)
    wo = np.asarray(wo, np.float32)

    cosT = np.ascontiguousarray(np.asarray(freqs_cos, np.float32).T)  # [32, S]
    sinT = np.ascontiguousarray(np.asarray(freqs_sin, np.float32).T)
    kcs = np.ascontiguousarray(np.concatenate([cosT, sinT], axis=0))  # [64, S]

    # constant diagonal mask [128, 2, 384]: m[k, :, i*128 + q] = (k <= q)
    tri = (np.arange(128)[:, None] <= np.arange(128)[None, :])
    maskd = np.broadcast_to(tri[:, None, None, :],
                            (128, 2, 3, 128)).reshape(128, 2, 384)
    maskd = np.ascontiguousarray(maskd.astype(np.float32))

    in_maps = []
    for core in range(8):
        b, half = core // 2, core % 2
        xT = np.ascontiguousarray(np.asarray(x[b], np.float32).T)
        qh = slice(half * HL * HD, (half + 1) * HL * HD)
        kh = slice(half * KVL * HD, (half + 1) * KVL * HD)
        in_maps.append({
            "xT": xT.astype(bf),
            "wq": _permute_cols(wq[:, qh], HL).astype(bf),
            "wk": _permute_cols(wk[:, kh], KVL).astype(bf),
            "wv": np.ascontiguousarray(wv[:, kh]).astype(bf),
            "wo": np.ascontiguousarray(wo[qh, :]).astype(bf),
            "kcs": kcs.astype(bf),
            "maskd": maskd.astype(bf),
        })
    return in_maps


_NC_CACHE = {}


def kernel(x, wq, wk, wv, wo, freqs_cos, freqs_sin, mask_attention,
           start_pos=0, inference=0, **_ignored):
    from concourse.bass_utils import run_bass_kernel_spmd

    in_maps = make_in_maps(np.asarray(x, np.float32), wq, wk, wv, wo,
                           freqs_cos, freqs_sin)
    if "nc" not in _NC_CACHE:
        _NC_CACHE["nc"] = build_nc()
    nc = _NC_CACHE["nc"]
    res = run_bass_kernel_spmd(nc, in_maps, core_ids=list(range(8)))
    outs = res.results
    out_full = np.zeros((B, S, D), np.float32)
    for b in range(B):
        z0 = np.asarray(outs[2 * b]["out"], np.float32)
        z1 = np.asarray(outs[2 * b + 1]["out"], np.float32)
        out_full[b] = (z0 + z1).T
    return out_full
